# revision 31
# baseline (speedup 1.0000x reference)
"""Distributed DMPNN kernel for 8 TRN2 NeuronCores (self-contained).

Sharding: molecules (hence atoms + their in-edges) partitioned across 8
cores; edges sorted by tgt and packed into fixed groups of 512 edge
slots covering <=128 whole tgt atoms. Per message pass: indirect-DMA
gather of per-src hidden rows, DVE add+relu, one-hot-matmul scatter
(segment sum) into PSUM, dense W_h matmul, chunked AllGather of the
staged hidden tables (overlapped with the producing pass). Readout uses
the same one-hot machinery over molecules. Matmuls in bf16, PSUM fp32.
"""
import os

os.environ.setdefault("MYCRO_LOCAL_CACHE", "1")

import numpy as np
import ml_dtypes

NCORES = 8
HIDDEN = 300
ATOM_F = 133
ATOM_FPAD = 136
BOND_F = 14
BOND_FPAD = 16
SLOTS = 512
WMAX = 128
PAD_TGT = 200.0
G2SLOTS = 512
NUM_MOLS = 4096

P = 128
H3 = [128, 128, 44]
H3OFF = [0, 128, 256]


# ----------------------------------------------------------------------
# host prep (index manipulation / input layout only)
# ----------------------------------------------------------------------

def _pack_atoms(deg, emax, wmax):
    groups = []
    n = len(deg)
    i = 0
    while i < n:
        e = 0
        j = i
        while j < n and (j - i) < wmax:
            if e + deg[j] > emax:
                break
            e += deg[j]
            j += 1
        assert j > i, f"atom {i} degree {deg[i]} > {emax}"
        groups.append((i, j))
        i = j
    return groups


def prep(x, edge_index, edge_attr, batch):
    N = x.shape[0]
    src_g = np.asarray(edge_index[0], dtype=np.int64)
    tgt_g = np.asarray(edge_index[1], dtype=np.int64)
    batch = np.asarray(batch, dtype=np.int64)
    edge_attr = np.asarray(edge_attr, dtype=np.float32)
    x = np.asarray(x, dtype=np.float32)
    mpc = NUM_MOLS // NCORES

    bounds = np.searchsorted(batch, np.arange(0, NUM_MOLS + 1, mpc))
    tgt_core = np.searchsorted(bounds, tgt_g, side="right") - 1

    cores = []
    for c in range(NCORES):
        alo, ahi = int(bounds[c]), int(bounds[c + 1])
        nat = ahi - alo
        eids = np.where(tgt_core == c)[0]
        eids = eids[np.argsort(tgt_g[eids], kind="stable")]
        tgt_l = tgt_g[eids] - alo
        deg = np.bincount(tgt_l, minlength=nat)
        groups = _pack_atoms(deg, SLOTS, WMAX)
        cores.append(dict(alo=alo, nat=nat, eids=eids, tgt_l=tgt_l, deg=deg,
                          groups=groups))

    G = max(len(cc["groups"]) for cc in cores)
    G = ((G + 12) // 13) * 13
    # allgather chunk sizes (in groups): big chunks first, small final chunk
    # so the exposed post-pass tail is short
    ch_sizes = [14] * (G // 14)
    if G % 14:
        ch_sizes.append(G % 14)
    ch_start = np.concatenate([[0], np.cumsum(ch_sizes)]).astype(np.int64)
    ch_row0 = NCORES * P * ch_start  # global row offset of each chunk

    def pack_mols(mol_deg):
        g2_list = []
        i = 0
        while i < mpc:
            a = 0
            j = i
            while j < mpc and (j - i) < P:
                if a + mol_deg[j] > G2SLOTS:
                    break
                a += mol_deg[j]
                j += 1
            assert j > i
            g2_list.append((i, j))
            i = j
        return g2_list

    for c, cc in enumerate(cores):
        mol_l = batch[cc["alo"]:cc["alo"] + cc["nat"]] - c * mpc
        cc["mol_l"] = mol_l
        cc["mol_deg"] = np.bincount(mol_l, minlength=mpc)
        cc["g2_list"] = pack_mols(cc["mol_deg"])
    G2 = max(len(cc["g2_list"]) for cc in cores)

    # staged row of each atom in the chunk-major all-gathered table
    g_chunk = np.searchsorted(ch_start, np.arange(G), side="right") - 1
    staged_of_atom = np.full(N, -1, dtype=np.int64)
    for c, cc in enumerate(cores):
        for g, (ws, we) in enumerate(cc["groups"]):
            a = np.arange(ws, we)
            ch = int(g_chunk[g])
            staged_of_atom[cc["alo"] + a] = (
                ch_row0[ch] + c * (ch_sizes[ch] * P)
                + (g - int(ch_start[ch])) * P + (a - ws))
    assert (staged_of_atom >= 0).all()

    per_core = []
    for c, cc in enumerate(cores):
        alo, nat, eids, tgt_l, groups = (cc["alo"], cc["nat"], cc["eids"],
                                         cc["tgt_l"], cc["groups"])
        ngroups = len(groups)
        tgt_local = np.full((G, SLOTS), PAD_TGT, dtype=np.float32)
        xsrc = np.zeros((G, SLOTS), dtype=np.int32)
        gsrc = np.zeros((G, SLOTS), dtype=np.int32)
        eaT = np.zeros((G, BOND_FPAD, SLOTS), dtype=np.float32)
        xT = np.zeros((G, ATOM_FPAD, P), dtype=np.float32)
        epos = 0
        for g, (ws, we) in enumerate(groups):
            ne = int(cc["deg"][ws:we].sum())
            sl = eids[epos:epos + ne]
            tgt_local[g, :ne] = (tgt_l[epos:epos + ne] - ws).astype(np.float32)
            xsrc[g, :ne] = src_g[sl].astype(np.int32)
            gsrc[g, :ne] = staged_of_atom[src_g[sl]].astype(np.int32)
            eaT[g][:BOND_F][:, :ne] = edge_attr[sl].T
            xT[g][:ATOM_F][:, :we - ws] = x[alo + ws:alo + we].T
            epos += ne
        assert epos == len(eids)

        # molecule packing for readout (precomputed above)
        mol_l = cc["mol_l"]
        mol_deg = cc["mol_deg"]
        g2_list = cc["g2_list"]
        assert len(g2_list) <= G2, (len(g2_list), G2)

        mol_local = np.full((G2, G2SLOTS), PAD_TGT, dtype=np.float32)
        mol_flush = np.full((G2, P), 512, dtype=np.int32)
        mol_trash = 512 + np.arange(P, dtype=np.int32)
        mol_astart = np.concatenate([[0], np.cumsum(mol_deg)])
        ah_row_of_atom = np.full(nat, -1, dtype=np.int64)
        for g2, (mi, mj) in enumerate(g2_list):
            a0, a1 = int(mol_astart[mi]), int(mol_astart[mj])
            assert a1 - a0 <= G2SLOTS
            mol_local[g2, :a1 - a0] = (mol_l[a0:a1] - mi).astype(np.float32)
            k = mj - mi
            mol_flush[g2, :k] = np.arange(mi, mj, dtype=np.int32)
            mol_flush[g2, k:] = mol_trash[k:]
            ah_row_of_atom[a0:a1] = g2 * G2SLOTS + np.arange(a1 - a0)
        for g2 in range(len(g2_list), G2):
            mol_flush[g2] = mol_trash
        assert (ah_row_of_atom >= 0).all()

        NA = G2 * G2SLOTS
        ah_flush = np.full((G, P), NA, dtype=np.int32)
        trash = NA + np.arange(P, dtype=np.int32)
        for g, (ws, we) in enumerate(groups):
            k = we - ws
            ah_flush[g, :k] = ah_row_of_atom[ws:we].astype(np.int32)
            ah_flush[g, k:] = trash[k:]
        for g in range(ngroups, G):
            ah_flush[g] = trash

        # device layouts:
        #  edge slot s=(j,p)=j*128+p: idx tables resident as [P, G*4]
        #  atom slot s2=(p,j)=p*4+j: mol table [P, G2*4]
        def pf(a):
            # [G, S] -> [P, G, 4]: arr[p, g, j] = val[g, j*128+p]
            return np.ascontiguousarray(
                a.reshape(G, 4, P).transpose(2, 0, 1))

        per_core.append(dict(
            tgt_pf=pf(tgt_local), xsrc_pf=pf(xsrc), gsrc_pf=pf(gsrc),
            eaT=eaT.astype(ml_dtypes.bfloat16),
            xT0=np.ascontiguousarray(
                xT[:, 0:P, :].transpose(1, 0, 2)).astype(ml_dtypes.bfloat16),
            xT1=np.ascontiguousarray(
                xT[:, P:ATOM_FPAD, :].transpose(1, 0, 2)).astype(ml_dtypes.bfloat16),
            ah_flush=np.ascontiguousarray(ah_flush.T),
            mol_pf=np.ascontiguousarray(
                mol_local.reshape(G2, P, 4).transpose(1, 0, 2)),
            mol_flush=np.ascontiguousarray(mol_flush.T),
        ))

    min_nat = min(cc["nat"] for cc in cores)
    meta = dict(G=G, G2=G2, min_nat=min_nat, N=N,
                ch_sizes=[int(v) for v in ch_sizes])
    return per_core, meta


# ----------------------------------------------------------------------
# device kernel builder
# ----------------------------------------------------------------------

def build_kernel(G, G2, memset_start, n_atoms, ch_sizes=None, debug=False):
    import contextlib
    import concourse.bass as bass
    import concourse.mybir as mybir
    import concourse.tile as tile
    from concourse import bacc

    fp32 = mybir.dt.float32
    bf16 = mybir.dt.bfloat16
    i32 = mybir.dt.int32
    AO = mybir.AluOpType
    H = HIDDEN
    if ch_sizes is None:
        ch_sizes = [13] * (G // 13)
    NCH = len(ch_sizes)
    ch_start = [0]
    for s_ in ch_sizes:
        ch_start.append(ch_start[-1] + s_)
    assert ch_start[-1] == G
    ch_row0 = [NCORES * P * v for v in ch_start]
    TROWS = NCORES * P * G
    NA = G2 * G2SLOTS

    nc = bacc.Bacc("TRN2", target_bir_lowering=False, debug=False,
                   num_devices=NCORES)

    def param(name, shape, dt=fp32, out=False):
        return nc.declare_dram_parameter(name, shape, dt, isOutput=out)

    x_pad = param("x_pad", [n_atoms, ATOM_FPAD], mybir.dt.bfloat16)
    tgt_pf = param("tgt_pf", [P, G * 4])
    xsrc_pf = param("xsrc_pf", [P, G * 4], i32)
    gsrc_pf = param("gsrc_pf", [P, G * 4], i32)
    eaT_in = param("eaT", [G, BOND_FPAD, SLOTS], mybir.dt.bfloat16)
    xT0_in = param("xT0", [P, G * P], mybir.dt.bfloat16)
    xT1_in = param("xT1", [ATOM_FPAD - P, G * P], mybir.dt.bfloat16)
    ahf_i = param("ahf_i", [P, G], i32)
    molf_i = param("molf_i", [P, G2], i32)
    mol_pf_in = param("mol_pf", [P, G2 * 4])
    iota_in = param("iota", [P, P])
    ident_in = param("ident", [P, P])
    wix = param("wix", [ATOM_FPAD, H])
    wie = param("wie", [BOND_FPAD, H])
    wxe = param("wxe", [48, H])
    whT = param("whT", [H, H])
    wox = param("wox", [ATOM_FPAD, H])
    womT = param("womT", [H, H])
    wob = param("wob", [P, H])
    f1T = param("f1T", [H, H])
    f1b = param("f1b", [P, H])
    f2T = param("f2T", [H, 1])
    f2b = param("f2b", [P, 1])
    out_ext = param("out", [512, 1], out=True)
    if debug:
        bf16_ = mybir.dt.bfloat16
        dbg_m1 = param("dbg_m1", [G, P, 4, HIDDEN], bf16_, out=True)
        dbg_m2 = param("dbg_m2", [G, P, 4, HIDDEN], bf16_, out=True)
        NCH_ = G // 8
        dbg_tab1 = param("dbg_tab1", [NCH_ * NCORES * 1024, HIDDEN], bf16_, out=True)
        dbg_tab2 = param("dbg_tab2", [NCH_ * NCORES * 1024, HIDDEN], bf16_, out=True)
        dbg_ah = param("dbg_ah", [G2 * G2SLOTS + 512, HIDDEN], bf16_, out=True)
        dbg_mol = param("dbg_mol", [512 + P, HIDDEN], bf16_, out=True)

    m_dram = nc.dram_tensor("m_dram", [G, P, 4, H], bf16)
    hl1 = [nc.dram_tensor(f"hl1_{c}", [ch_sizes[c] * P, H], bf16)
           for c in range(NCH)]
    hl2 = [nc.dram_tensor(f"hl2_{c}", [ch_sizes[c] * P, H], bf16)
           for c in range(NCH)]
    g2chunk = []
    for ci, s_ in enumerate(ch_sizes):
        g2chunk += [ci] * s_
    tab1 = nc.dram_tensor("tab1", [TROWS, H], bf16, addr_space="Shared")
    tab2 = nc.dram_tensor("tab2", [TROWS, H], bf16, addr_space="Shared")
    ah_tab = nc.dram_tensor("ah_tab", [NA + 512, H], bf16)
    mol_tab = nc.dram_tensor("mol_tab", [512 + P, H], bf16)

    RG = [list(range(NCORES))]

    with contextlib.ExitStack() as ctx:
        tc = ctx.enter_context(tile.TileContext(nc))
        cpool = ctx.enter_context(tc.tile_pool(name="const", bufs=1))
        sb = ctx.enter_context(tc.tile_pool(name="sb", bufs=4))
        sb6 = ctx.enter_context(tc.tile_pool(name="sb6", bufs=5))
        pp = ctx.enter_context(tc.tile_pool(name="psum", bufs=2, space="PSUM"))

        # ---- constants ----
        def cast_const(name, src, rows, cols, dt=bf16):
            t32 = cpool.tile([rows, cols], fp32, tag=name + "_32", name=name + "_32")
            nc.sync.dma_start(out=t32[:], in_=src)
            tb = cpool.tile([rows, cols], dt, tag=name, name=name)
            nc.vector.tensor_copy(tb[:], t32[:])
            return tb

        def load_const(name, src, rows, cols):
            t = cpool.tile([rows, cols], fp32, tag=name, name=name)
            nc.sync.dma_start(out=t[:], in_=src)
            return t

        wix_b0 = cast_const("wix0", wix[0:104, :], 104, H)
        wie_b = cast_const("wie", wie[:], BOND_FPAD, H)
        wxe_b = cast_const("wxe", wxe[:], 48, H)
        whT_b = [cast_const(f"wh{ci}", whT[H3OFF[ci]:H3OFF[ci] + H3[ci], :],
                            H3[ci], H) for ci in range(3)]
        wox_b0 = cast_const("wox0", wox[0:P, :], P, H)
        wox_b1 = cast_const("wox1", wox[P:ATOM_FPAD, :], ATOM_FPAD - P, H)
        womT_b = [cast_const(f"wom{ci}", womT[H3OFF[ci]:H3OFF[ci] + H3[ci], :],
                             H3[ci], H) for ci in range(3)]
        f1T_b = [cast_const(f"f1_{ci}", f1T[H3OFF[ci]:H3OFF[ci] + H3[ci], :],
                            H3[ci], H) for ci in range(3)]
        f2T_b = [cast_const(f"f2_{ci}", f2T[H3OFF[ci]:H3OFF[ci] + H3[ci], :],
                            H3[ci], 1) for ci in range(3)]
        wob_t = load_const("wob", wob[:], P, H)
        f1b_t = load_const("f1b", f1b[:], P, H)
        f2b_t = load_const("f2b", f2b[:], P, 1)
        iota_t = load_const("iota", iota_in[:], P, P)
        ident_f = load_const("identf", ident_in[:], P, P)
        ident_b = cast_const("identb", ident_in[:], P, P)

        tgt_all = load_const("tgt_all", tgt_pf[:], P, G * 4)
        gsrc_all = cpool.tile([P, G * 4], i32, tag="gsrc_all", name="gsrc_all")
        nc.sync.dma_start(out=gsrc_all[:], in_=gsrc_pf[:])
        xsrc_all = cpool.tile([P, G * 4], i32, tag="xsrc_all", name="xsrc_all")
        nc.sync.dma_start(out=xsrc_all[:], in_=xsrc_pf[:])
        ahf_all = cpool.tile([P, G], i32, tag="ahf_all", name="ahf_all")
        nc.sync.dma_start(out=ahf_all[:], in_=ahf_i[:])
        molf_all = cpool.tile([P, G2], i32, tag="molf_all", name="molf_all")
        nc.sync.dma_start(out=molf_all[:], in_=molf_i[:])
        molpf_all = load_const("molpf_all", mol_pf_in[:], P, G2 * 4)
        xt0_all = cpool.tile([P, G * P], bf16, tag="xt0_all", name="xt0_all")
        nc.sync.dma_start(out=xt0_all[:], in_=xT0_in[:])
        xt1_all = cpool.tile([ATOM_FPAD - P, G * P], bf16, tag="xt1_all",
                             name="xt1_all")
        nc.sync.dma_start(out=xt1_all[:], in_=xT1_in[:])

        # ---- memset tail of ah_tab (uninit DRAM must not feed matmuls) ----
        zero_b = cpool.tile([P, H], bf16, tag="zeros")
        nc.vector.memset(zero_b[:], 0.0)
        r = memset_start
        while r < NA + 512:
            n = min(P, NA + 512 - r)
            nc.sync.dma_start(out=ah_tab[r:r + n, :], in_=zero_b[0:n, :])
            r += n

        def onehot(idx_col, tag):
            s = sb.tile([P, P], bf16, tag=tag)
            nc.vector.tensor_tensor(out=s[:], in0=idx_col.to_broadcast([P, P]),
                                    in1=iota_t[:], op=AO.is_equal)
            return s

        def scatter_mms(neiT_ps, m_slice, s_tile, j):
            # one PSUM bank: start clears has_written BANK-wide, so only the
            # very first matmul of the group may set it (and one stop at end)
            for ci in range(3):
                nc.tensor.matmul(
                    out=neiT_ps[0:H3[ci], P * ci:P * ci + P],
                    lhsT=m_slice[:, H3OFF[ci]:H3OFF[ci] + H3[ci]],
                    rhs=s_tile[:],
                    start=(j == 0 and ci == 0), stop=(j == 3 and ci == 2))

        def dense_from_neiT(neiT_ps, wchunks, extra=()):
            neiT_sb = sb.tile([P, 3 * P], bf16, tag="neiTsb")
            nc.vector.tensor_copy(neiT_sb[:], neiT_ps[:])
            hps = pp.tile([P, H], fp32, space="PSUM", tag="hnei")
            nmm = 3 + len(extra)
            k = 0
            for lhsT, rhs in extra:
                nc.tensor.matmul(out=hps[:], lhsT=lhsT, rhs=rhs,
                                 start=(k == 0), stop=(k == nmm - 1))
                k += 1
            for ci in range(3):
                nc.tensor.matmul(
                    out=hps[:],
                    lhsT=neiT_sb[0:H3[ci], P * ci:P * ci + P],
                    rhs=wchunks[ci][:],
                    start=(k == 0), stop=(k == nmm - 1))
                k += 1
            return hps

        def relu_to(dst_ap, src_ap):
            nc.scalar.activation(out=dst_ap, in_=src_ap,
                                 func=mybir.ActivationFunctionType.Relu)

        # ================= PASS 1 =================
        for g in range(G):
            ea_b = sb.tile([BOND_FPAD, SLOTS], bf16, tag="eab")
            nc.sync.dma_start(out=ea_b[:], in_=eaT_in[g])

            m_bf = sb.tile([P, 4, H], bf16, tag="mbf")
            neiT_ps = pp.tile([P, 3 * P], fp32, space="PSUM", tag="neiT")
            for j in range(4):
                gx = sb.tile([P, ATOM_FPAD], bf16, tag="gx")
                nc.gpsimd.indirect_dma_start(
                    out=gx[:], out_offset=None, in_=x_pad[:],
                    in_offset=bass.IndirectOffsetOnAxis(
                        ap=xsrc_all[:, g * 4 + j:g * 4 + j + 1], axis=0))
                tps = pp.tile([P, 2 * P], bf16, space="PSUM", tag="tps")
                nc.tensor.transpose(out=tps[0:104, 0:P], in_=gx[:, 0:104],
                                    identity=ident_b[:])
                nc.tensor.transpose(out=tps[0:32, P:2 * P],
                                    in_=gx[:, 104:ATOM_FPAD],
                                    identity=ident_b[:])
                xt_b = sb.tile([104, P], bf16, tag="xtb")
                nc.vector.tensor_copy(xt_b[:], tps[0:104, 0:P])
                xc_b = sb.tile([48, P], bf16, tag="xcb")
                nc.vector.tensor_copy(xc_b[0:32, :], tps[0:32, P:2 * P])
                nc.vector.tensor_copy(xc_b[32:48, :], ea_b[:, P * j:P * j + P])
                m0ps = pp.tile([P, H], fp32, space="PSUM", tag="m0")
                nc.tensor.matmul(out=m0ps[:], lhsT=xt_b[:],
                                 rhs=wix_b0[:], start=True, stop=False)
                nc.tensor.matmul(out=m0ps[:], lhsT=xc_b[:],
                                 rhs=wxe_b[:], start=False, stop=True)
                relu_to(m_bf[:, j, :], m0ps[:])
                s = onehot(tgt_all[:, g * 4 + j:g * 4 + j + 1], "s1")
                scatter_mms(neiT_ps, m_bf[:, j, :], s, j)
            nc.sync.dma_start(out=m_dram[g], in_=m_bf[:])
            hps = dense_from_neiT(neiT_ps, whT_b)
            hl_sb = sb.tile([P, H], bf16, tag="hlsb")
            nc.scalar.activation(out=hl_sb[:], in_=hps[:],
                                 func=mybir.ActivationFunctionType.Copy)
            c = g2chunk[g]
            go = g - ch_start[c]
            nc.sync.dma_start(out=hl1[c][go * P:(go + 1) * P, :],
                              in_=hl_sb[:])
            if g == ch_start[c + 1] - 1:
                nc.gpsimd.collective_compute(
                    "AllGather", AO.bypass, replica_groups=RG,
                    ins=[hl1[c][:]],
                    outs=[tab1[ch_row0[c]:ch_row0[c]
                                + NCORES * ch_sizes[c] * P, :]])

        if debug:
            for g in range(G):
                nc.sync.dma_start(out=dbg_m1[g], in_=m_dram[g])
            nc.sync.dma_start(out=dbg_tab1[:], in_=tab1[:])

        # ================= PASS 2 =================
        for g in range(G):
            m_in = sb.tile([P, 4, H], bf16, tag="min")
            nc.sync.dma_start(out=m_in[:], in_=m_dram[g])
            m_bf = sb.tile([P, 4, H], bf16, tag="mbf2")
            gth = sb6.tile([P, 4, H], bf16, tag="gth")
            neiT_ps = pp.tile([P, 3 * P], fp32, space="PSUM", tag="neiT")
            for j in range(4):
                nc.gpsimd.indirect_dma_start(
                    out=gth[:, j, :], out_offset=None, in_=tab1[:],
                    in_offset=bass.IndirectOffsetOnAxis(
                        ap=gsrc_all[:, g * 4 + j:g * 4 + j + 1], axis=0))
                tmp = sb.tile([P, H], fp32, tag="tmp2")
                nc.vector.tensor_tensor(out=tmp[:], in0=m_in[:, j, :],
                                        in1=gth[:, j, :], op=AO.add)
                relu_to(m_bf[:, j, :], tmp[:])
                s = onehot(tgt_all[:, g * 4 + j:g * 4 + j + 1], "s2")
                scatter_mms(neiT_ps, m_bf[:, j, :], s, j)
            nc.sync.dma_start(out=m_dram[g], in_=m_bf[:])
            hps = dense_from_neiT(neiT_ps, whT_b)
            hl_sb = sb.tile([P, H], bf16, tag="hlsb2")
            nc.scalar.activation(out=hl_sb[:], in_=hps[:],
                                 func=mybir.ActivationFunctionType.Copy)
            c = g2chunk[g]
            go = g - ch_start[c]
            nc.sync.dma_start(out=hl2[c][go * P:(go + 1) * P, :],
                              in_=hl_sb[:])
            if g == ch_start[c + 1] - 1:
                nc.gpsimd.collective_compute(
                    "AllGather", AO.bypass, replica_groups=RG,
                    ins=[hl2[c][:]],
                    outs=[tab2[ch_row0[c]:ch_row0[c]
                                + NCORES * ch_sizes[c] * P, :]])

        if debug:
            for g in range(G):
                nc.sync.dma_start(out=dbg_m2[g], in_=m_dram[g])
            nc.sync.dma_start(out=dbg_tab2[:], in_=tab2[:])

        # ================= PASS 3 + atom readout =================
        for g in range(G):
            m_in = sb6.tile([P, 4, H], bf16, tag="min3")
            nc.sync.dma_start(out=m_in[:], in_=m_dram[g])
            msgT_ps = pp.tile([P, 3 * P], fp32, space="PSUM", tag="neiT")
            for j in range(4):
                gth = sb6.tile([P, H], bf16, tag="gth3")
                nc.gpsimd.indirect_dma_start(
                    out=gth[:], out_offset=None, in_=tab2[:],
                    in_offset=bass.IndirectOffsetOnAxis(
                        ap=gsrc_all[:, g * 4 + j:g * 4 + j + 1], axis=0))
                tmp = sb.tile([P, H], fp32, tag="tmp3")
                nc.vector.tensor_tensor(out=tmp[:], in0=m_in[:, j, :],
                                        in1=gth[:], op=AO.add)
                m2 = sb.tile([P, H], bf16, tag="m2")
                relu_to(m2[:], tmp[:])
                s = onehot(tgt_all[:, g * 4 + j:g * 4 + j + 1], "s3")
                scatter_mms(msgT_ps, m2[:], s, j)
            hps = dense_from_neiT(
                msgT_ps, womT_b,
                extra=[(xt0_all[:, g * P:(g + 1) * P], wox_b0[:]),
                       (xt1_all[:, g * P:(g + 1) * P], wox_b1[:])])
            tmp = sb.tile([P, H], fp32, tag="tmpah")
            nc.vector.tensor_tensor(out=tmp[:], in0=hps[:], in1=wob_t[:],
                                    op=AO.add)
            ah_sb = sb.tile([P, H], bf16, tag="ahsb")
            relu_to(ah_sb[:], tmp[:])
            nc.gpsimd.indirect_dma_start(
                out=ah_tab[:],
                out_offset=bass.IndirectOffsetOnAxis(
                    ap=ahf_all[:, g:g + 1], axis=0),
                in_=ah_sb[:], in_offset=None)

        if debug:
            nc.sync.dma_start(out=dbg_ah[:], in_=ah_tab[:])

        # ================= molecule reduction =================
        for g2 in range(G2):
            ah_in = sb.tile([P, 4, H], bf16, tag="ahin")
            nc.sync.dma_start(
                out=ah_in[:],
                in_=ah_tab[g2 * G2SLOTS:(g2 + 1) * G2SLOTS, :].rearrange(
                    "(p j) h -> p j h", j=4))
            mol_ps = pp.tile([P, H], fp32, space="PSUM", tag="hnei")
            for j in range(4):
                s = onehot(molpf_all[:, g2 * 4 + j:g2 * 4 + j + 1], "sm")
                nc.tensor.matmul(out=mol_ps[:], lhsT=s[:], rhs=ah_in[:, j, :],
                                 start=(j == 0), stop=(j == 3))
            mol_sb = sb.tile([P, H], bf16, tag="molsb")
            nc.vector.tensor_copy(mol_sb[:], mol_ps[:])
            nc.gpsimd.indirect_dma_start(
                out=mol_tab[:],
                out_offset=bass.IndirectOffsetOnAxis(
                    ap=molf_all[:, g2:g2 + 1], axis=0),
                in_=mol_sb[:], in_offset=None)

        if debug:
            nc.sync.dma_start(out=dbg_mol[:], in_=mol_tab[:])

        # ================= FFN =================
        for t in range(4):
            mt = sb.tile([P, H], bf16, tag="mt")
            nc.sync.dma_start(out=mt[:], in_=mol_tab[t * P:(t + 1) * P, :])
            hT = sb.tile([P, 3 * P], bf16, tag="hT")
            for ci in range(3):
                tp = pp.tile([P, P], bf16, space="PSUM", tag="tps")
                nc.tensor.transpose(out=tp[0:H3[ci], 0:P],
                                    in_=mt[:, H3OFF[ci]:H3OFF[ci] + H3[ci]],
                                    identity=ident_b[:])
                nc.vector.tensor_copy(hT[:, P * ci:P * ci + P], tp[:, 0:P])
            hps = pp.tile([P, H], fp32, space="PSUM", tag="hnei")
            for ci in range(3):
                nc.tensor.matmul(out=hps[:],
                                 lhsT=hT[0:H3[ci], P * ci:P * ci + P],
                                 rhs=f1T_b[ci][:], start=(ci == 0),
                                 stop=(ci == 2))
            tmp = sb.tile([P, H], fp32, tag="tmpf")
            nc.vector.tensor_tensor(out=tmp[:], in0=hps[:], in1=f1b_t[:],
                                    op=AO.add)
            h_sb = sb.tile([P, H], bf16, tag="hsb")
            relu_to(h_sb[:], tmp[:])
            h2T = sb.tile([P, 3 * P], bf16, tag="h2T")
            for ci in range(3):
                tp = pp.tile([P, P], bf16, space="PSUM", tag="tps")
                nc.tensor.transpose(out=tp[0:H3[ci], 0:P],
                                    in_=h_sb[:, H3OFF[ci]:H3OFF[ci] + H3[ci]],
                                    identity=ident_b[:])
                nc.vector.tensor_copy(h2T[:, P * ci:P * ci + P], tp[:, 0:P])
            ops = pp.tile([P, 1], fp32, space="PSUM", tag="m0")
            for ci in range(3):
                nc.tensor.matmul(out=ops[:],
                                 lhsT=h2T[0:H3[ci], P * ci:P * ci + P],
                                 rhs=f2T_b[ci][:], start=(ci == 0),
                                 stop=(ci == 2))
            o_sb = sb.tile([P, 1], fp32, tag="osb")
            nc.vector.tensor_tensor(out=o_sb[:], in0=ops[:], in1=f2b_t[:],
                                    op=AO.add)
            nc.sync.dma_start(out=out_ext[t * P:(t + 1) * P, :], in_=o_sb[:])

    nc.compile()
    return nc


# ----------------------------------------------------------------------
# entry point
# ----------------------------------------------------------------------

def _build_in_maps(inputs, per_core, meta):
    x = np.asarray(inputs["x"], dtype=np.float32)
    N = x.shape[0]
    x_pad = np.zeros((N, ATOM_FPAD), dtype=ml_dtypes.bfloat16)
    x_pad[:, :ATOM_F] = x.astype(ml_dtypes.bfloat16)

    W_i = np.asarray(inputs["W_i"], dtype=np.float32)
    W_h = np.asarray(inputs["W_h"], dtype=np.float32)
    W_o_w = np.asarray(inputs["W_o_w"], dtype=np.float32)
    W_o_b = np.asarray(inputs["W_o_b"], dtype=np.float32)
    f1w = np.asarray(inputs["ffn1_w"], dtype=np.float32)
    f1b = np.asarray(inputs["ffn1_b"], dtype=np.float32)
    f2w = np.asarray(inputs["ffn2_w"], dtype=np.float32)
    f2b = np.asarray(inputs["ffn2_b"], dtype=np.float32)

    wix = np.zeros((ATOM_FPAD, HIDDEN), np.float32)
    wix[:ATOM_F] = W_i[:, :ATOM_F].T
    wie = np.zeros((BOND_FPAD, HIDDEN), np.float32)
    wie[:BOND_F] = W_i[:, ATOM_F:].T
    wxe = np.zeros((48, HIDDEN), np.float32)
    wxe[:32] = wix[104:ATOM_FPAD]
    wxe[32:32 + BOND_F] = W_i[:, ATOM_F:].T
    wox = np.zeros((ATOM_FPAD, HIDDEN), np.float32)
    wox[:ATOM_F] = W_o_w[:, :ATOM_F].T
    shared = dict(
        x_pad=x_pad,
        iota=np.tile(np.arange(P, dtype=np.float32), (P, 1)),
        ident=np.eye(P, dtype=np.float32),
        wix=wix, wie=wie, wxe=wxe,
        whT=np.ascontiguousarray(W_h.T),
        wox=wox,
        womT=np.ascontiguousarray(W_o_w[:, ATOM_F:].T),
        wob=np.tile(W_o_b[None, :], (P, 1)).astype(np.float32),
        f1T=np.ascontiguousarray(f1w.T),
        f1b=np.tile(f1b[None, :], (P, 1)).astype(np.float32),
        f2T=np.ascontiguousarray(f2w.T),
        f2b=np.full((P, 1), float(f2b[0]), np.float32),
    )
    in_maps = []
    for c in range(NCORES):
        pc = per_core[c]
        m = dict(shared)
        G, G2 = meta["G"], meta["G2"]
        m.update(
            tgt_pf=pc["tgt_pf"].reshape(P, G * 4),
            xsrc_pf=pc["xsrc_pf"].reshape(P, G * 4),
            gsrc_pf=pc["gsrc_pf"].reshape(P, G * 4),
            eaT=pc["eaT"],
            xT0=pc["xT0"].reshape(P, G * P),
            xT1=pc["xT1"].reshape(ATOM_FPAD - P, G * P),
            ahf_i=pc["ah_flush"], molf_i=pc["mol_flush"],
            mol_pf=pc["mol_pf"].reshape(P, G2 * 4),
        )
        in_maps.append(m)
    return in_maps


_CACHED = {}


def kernel(profile=False, debug=False, **inputs):
    from concourse.bass_utils import run_bass_kernel_spmd

    per_core, meta = prep(inputs["x"], inputs["edge_index"],
                          inputs["edge_attr"], inputs["batch"])
    G, G2 = meta["G"], meta["G2"]
    # g2-slot pad rows are interspersed: zero the whole table
    memset_start = 0
    key = (G, G2, memset_start, meta["N"], tuple(meta["ch_sizes"]), debug)
    if key not in _CACHED:
        _CACHED[key] = build_kernel(G, G2, memset_start, meta["N"],
                                    ch_sizes=meta["ch_sizes"], debug=debug)
    nc = _CACHED[key]
    in_maps = _build_in_maps(inputs, per_core, meta)
    res = run_bass_kernel_spmd(nc, in_maps, core_ids=list(range(NCORES)),
                               trace=profile)
    out = np.concatenate([res.results[c]["out"] for c in range(NCORES)],
                         axis=0).astype(np.float32)
    if debug:
        return out, res.results
    if profile:
        return out, res.exec_time_ns
    return out


# revision 32
# speedup vs baseline: 1.0401x; 1.0401x over previous
"""Distributed DMPNN kernel for 8 TRN2 NeuronCores (self-contained).

Sharding: molecules (hence atoms + their in-edges) partitioned across 8
cores; edges sorted by tgt and packed into fixed groups of 512 edge
slots covering <=128 whole tgt atoms. Per message pass: indirect-DMA
gather of per-src hidden rows, DVE add+relu, one-hot-matmul scatter
(segment sum) into PSUM, dense W_h matmul, chunked AllGather of the
staged hidden tables (overlapped with the producing pass). Readout uses
the same one-hot machinery over molecules. Matmuls in bf16, PSUM fp32.
"""
import os

os.environ.setdefault("MYCRO_LOCAL_CACHE", "1")

import numpy as np
import ml_dtypes

NCORES = 8
HIDDEN = 300
ATOM_F = 133
ATOM_FPAD = 136
BOND_F = 14
BOND_FPAD = 16
SLOTS = 512
WMAX = 128
PAD_TGT = 200.0
G2SLOTS = 512
NUM_MOLS = 4096

P = 128
H3 = [128, 128, 44]
H3OFF = [0, 128, 256]


# ----------------------------------------------------------------------
# host prep (index manipulation / input layout only)
# ----------------------------------------------------------------------

def _pack_atoms(deg, emax, wmax):
    groups = []
    n = len(deg)
    i = 0
    while i < n:
        e = 0
        j = i
        while j < n and (j - i) < wmax:
            if e + deg[j] > emax:
                break
            e += deg[j]
            j += 1
        assert j > i, f"atom {i} degree {deg[i]} > {emax}"
        groups.append((i, j))
        i = j
    return groups


def prep(x, edge_index, edge_attr, batch):
    N = x.shape[0]
    src_g = np.asarray(edge_index[0], dtype=np.int64)
    tgt_g = np.asarray(edge_index[1], dtype=np.int64)
    batch = np.asarray(batch, dtype=np.int64)
    edge_attr = np.asarray(edge_attr, dtype=np.float32)
    x = np.asarray(x, dtype=np.float32)
    mpc = NUM_MOLS // NCORES

    bounds = np.searchsorted(batch, np.arange(0, NUM_MOLS + 1, mpc))
    tgt_core = np.searchsorted(bounds, tgt_g, side="right") - 1

    cores = []
    for c in range(NCORES):
        alo, ahi = int(bounds[c]), int(bounds[c + 1])
        nat = ahi - alo
        eids = np.where(tgt_core == c)[0]
        eids = eids[np.argsort(tgt_g[eids], kind="stable")]
        tgt_l = tgt_g[eids] - alo
        deg = np.bincount(tgt_l, minlength=nat)
        groups = _pack_atoms(deg, SLOTS, WMAX)
        cores.append(dict(alo=alo, nat=nat, eids=eids, tgt_l=tgt_l, deg=deg,
                          groups=groups))

    G = max(len(cc["groups"]) for cc in cores)
    G = ((G + 12) // 13) * 13
    # allgather chunk sizes (in groups): big chunks first, tapering tail so
    # the post-pass exposed collective chain is short
    if G == 104:
        ch_sizes = [16, 16, 14, 14, 14, 12, 10, 8]
    else:
        ch_sizes = [14] * (G // 14)
        if G % 14:
            ch_sizes.append(G % 14)
    ch_start = np.concatenate([[0], np.cumsum(ch_sizes)]).astype(np.int64)
    ch_row0 = NCORES * P * ch_start  # global row offset of each chunk

    def pack_mols(mol_deg):
        g2_list = []
        i = 0
        while i < mpc:
            a = 0
            j = i
            while j < mpc and (j - i) < P:
                if a + mol_deg[j] > G2SLOTS:
                    break
                a += mol_deg[j]
                j += 1
            assert j > i
            g2_list.append((i, j))
            i = j
        return g2_list

    for c, cc in enumerate(cores):
        mol_l = batch[cc["alo"]:cc["alo"] + cc["nat"]] - c * mpc
        cc["mol_l"] = mol_l
        cc["mol_deg"] = np.bincount(mol_l, minlength=mpc)
        cc["g2_list"] = pack_mols(cc["mol_deg"])
    G2 = max(len(cc["g2_list"]) for cc in cores)

    # staged row of each atom in the chunk-major all-gathered table
    g_chunk = np.searchsorted(ch_start, np.arange(G), side="right") - 1
    staged_of_atom = np.full(N, -1, dtype=np.int64)
    for c, cc in enumerate(cores):
        for g, (ws, we) in enumerate(cc["groups"]):
            a = np.arange(ws, we)
            ch = int(g_chunk[g])
            staged_of_atom[cc["alo"] + a] = (
                ch_row0[ch] + c * (ch_sizes[ch] * P)
                + (g - int(ch_start[ch])) * P + (a - ws))
    assert (staged_of_atom >= 0).all()

    per_core = []
    for c, cc in enumerate(cores):
        alo, nat, eids, tgt_l, groups = (cc["alo"], cc["nat"], cc["eids"],
                                         cc["tgt_l"], cc["groups"])
        ngroups = len(groups)
        tgt_local = np.full((G, SLOTS), PAD_TGT, dtype=np.float32)
        xsrc = np.zeros((G, SLOTS), dtype=np.int32)
        gsrc = np.zeros((G, SLOTS), dtype=np.int32)
        eaT = np.zeros((G, BOND_FPAD, SLOTS), dtype=np.float32)
        xT = np.zeros((G, ATOM_FPAD, P), dtype=np.float32)
        epos = 0
        for g, (ws, we) in enumerate(groups):
            ne = int(cc["deg"][ws:we].sum())
            sl = eids[epos:epos + ne]
            tgt_local[g, :ne] = (tgt_l[epos:epos + ne] - ws).astype(np.float32)
            xsrc[g, :ne] = src_g[sl].astype(np.int32)
            gsrc[g, :ne] = staged_of_atom[src_g[sl]].astype(np.int32)
            eaT[g][:BOND_F][:, :ne] = edge_attr[sl].T
            xT[g][:ATOM_F][:, :we - ws] = x[alo + ws:alo + we].T
            epos += ne
        assert epos == len(eids)

        # molecule packing for readout (precomputed above)
        mol_l = cc["mol_l"]
        mol_deg = cc["mol_deg"]
        g2_list = cc["g2_list"]
        assert len(g2_list) <= G2, (len(g2_list), G2)

        mol_local = np.full((G2, G2SLOTS), PAD_TGT, dtype=np.float32)
        mol_flush = np.full((G2, P), 512, dtype=np.int32)
        mol_trash = 512 + np.arange(P, dtype=np.int32)
        mol_astart = np.concatenate([[0], np.cumsum(mol_deg)])
        ah_row_of_atom = np.full(nat, -1, dtype=np.int64)
        for g2, (mi, mj) in enumerate(g2_list):
            a0, a1 = int(mol_astart[mi]), int(mol_astart[mj])
            assert a1 - a0 <= G2SLOTS
            mol_local[g2, :a1 - a0] = (mol_l[a0:a1] - mi).astype(np.float32)
            k = mj - mi
            mol_flush[g2, :k] = np.arange(mi, mj, dtype=np.int32)
            mol_flush[g2, k:] = mol_trash[k:]
            ah_row_of_atom[a0:a1] = g2 * G2SLOTS + np.arange(a1 - a0)
        for g2 in range(len(g2_list), G2):
            mol_flush[g2] = mol_trash
        assert (ah_row_of_atom >= 0).all()

        NA = G2 * G2SLOTS
        ah_flush = np.full((G, P), NA, dtype=np.int32)
        trash = NA + np.arange(P, dtype=np.int32)
        for g, (ws, we) in enumerate(groups):
            k = we - ws
            ah_flush[g, :k] = ah_row_of_atom[ws:we].astype(np.int32)
            ah_flush[g, k:] = trash[k:]
        for g in range(ngroups, G):
            ah_flush[g] = trash

        # device layouts:
        #  edge slot s=(j,p)=j*128+p: idx tables resident as [P, G*4]
        #  atom slot s2=(p,j)=p*4+j: mol table [P, G2*4]
        def pf(a):
            # [G, S] -> [P, G, 4]: arr[p, g, j] = val[g, j*128+p]
            return np.ascontiguousarray(
                a.reshape(G, 4, P).transpose(2, 0, 1))

        per_core.append(dict(
            tgt_pf=pf(tgt_local), xsrc_pf=pf(xsrc), gsrc_pf=pf(gsrc),
            eaT=eaT.astype(ml_dtypes.bfloat16),
            xT0=np.ascontiguousarray(
                xT[:, 0:P, :].transpose(1, 0, 2)).astype(ml_dtypes.bfloat16),
            xT1=np.ascontiguousarray(
                xT[:, P:ATOM_FPAD, :].transpose(1, 0, 2)).astype(ml_dtypes.bfloat16),
            ah_flush=np.ascontiguousarray(ah_flush.T),
            mol_pf=np.ascontiguousarray(
                mol_local.reshape(G2, P, 4).transpose(1, 0, 2)),
            mol_flush=np.ascontiguousarray(mol_flush.T),
        ))

    min_nat = min(cc["nat"] for cc in cores)
    meta = dict(G=G, G2=G2, min_nat=min_nat, N=N,
                ch_sizes=[int(v) for v in ch_sizes])
    return per_core, meta


# ----------------------------------------------------------------------
# device kernel builder
# ----------------------------------------------------------------------

def build_kernel(G, G2, memset_start, n_atoms, ch_sizes=None, debug=False):
    import contextlib
    import concourse.bass as bass
    import concourse.mybir as mybir
    import concourse.tile as tile
    from concourse import bacc

    fp32 = mybir.dt.float32
    bf16 = mybir.dt.bfloat16
    i32 = mybir.dt.int32
    AO = mybir.AluOpType
    H = HIDDEN
    if ch_sizes is None:
        ch_sizes = [13] * (G // 13)
    NCH = len(ch_sizes)
    ch_start = [0]
    for s_ in ch_sizes:
        ch_start.append(ch_start[-1] + s_)
    assert ch_start[-1] == G
    ch_row0 = [NCORES * P * v for v in ch_start]
    TROWS = NCORES * P * G
    NA = G2 * G2SLOTS

    nc = bacc.Bacc("TRN2", target_bir_lowering=False, debug=False,
                   num_devices=NCORES)

    def param(name, shape, dt=fp32, out=False):
        return nc.declare_dram_parameter(name, shape, dt, isOutput=out)

    x_pad = param("x_pad", [n_atoms, ATOM_FPAD], mybir.dt.bfloat16)
    tgt_pf = param("tgt_pf", [P, G * 4])
    xsrc_pf = param("xsrc_pf", [P, G * 4], i32)
    gsrc_pf = param("gsrc_pf", [P, G * 4], i32)
    eaT_in = param("eaT", [G, BOND_FPAD, SLOTS], mybir.dt.bfloat16)
    xT0_in = param("xT0", [P, G * P], mybir.dt.bfloat16)
    xT1_in = param("xT1", [ATOM_FPAD - P, G * P], mybir.dt.bfloat16)
    ahf_i = param("ahf_i", [P, G], i32)
    molf_i = param("molf_i", [P, G2], i32)
    mol_pf_in = param("mol_pf", [P, G2 * 4])
    iota_in = param("iota", [P, P])
    ident_in = param("ident", [P, P])
    wix = param("wix", [ATOM_FPAD, H])
    wie = param("wie", [BOND_FPAD, H])
    wxe = param("wxe", [48, H])
    whT = param("whT", [H, H])
    wox = param("wox", [ATOM_FPAD, H])
    womT = param("womT", [H, H])
    wob = param("wob", [P, H])
    f1T = param("f1T", [H, H])
    f1b = param("f1b", [P, H])
    f2T = param("f2T", [H, 1])
    f2b = param("f2b", [P, 1])
    out_ext = param("out", [512, 1], out=True)
    if debug:
        bf16_ = mybir.dt.bfloat16
        dbg_m1 = param("dbg_m1", [G, P, 4, HIDDEN], bf16_, out=True)
        dbg_m2 = param("dbg_m2", [G, P, 4, HIDDEN], bf16_, out=True)
        NCH_ = G // 8
        dbg_tab1 = param("dbg_tab1", [NCH_ * NCORES * 1024, HIDDEN], bf16_, out=True)
        dbg_tab2 = param("dbg_tab2", [NCH_ * NCORES * 1024, HIDDEN], bf16_, out=True)
        dbg_ah = param("dbg_ah", [G2 * G2SLOTS + 512, HIDDEN], bf16_, out=True)
        dbg_mol = param("dbg_mol", [512 + P, HIDDEN], bf16_, out=True)

    m_dram = nc.dram_tensor("m_dram", [G, P, 4, H], bf16)
    hl1 = [nc.dram_tensor(f"hl1_{c}", [ch_sizes[c] * P, H], bf16)
           for c in range(NCH)]
    hl2 = [nc.dram_tensor(f"hl2_{c}", [ch_sizes[c] * P, H], bf16)
           for c in range(NCH)]
    g2chunk = []
    for ci, s_ in enumerate(ch_sizes):
        g2chunk += [ci] * s_
    tab1 = nc.dram_tensor("tab1", [TROWS, H], bf16, addr_space="Shared")
    tab2 = nc.dram_tensor("tab2", [TROWS, H], bf16, addr_space="Shared")
    ah_tab = nc.dram_tensor("ah_tab", [NA + 512, H], bf16)
    mol_tab = nc.dram_tensor("mol_tab", [512 + P, H], bf16)

    RG = [list(range(NCORES))]

    with contextlib.ExitStack() as ctx:
        tc = ctx.enter_context(tile.TileContext(nc))
        cpool = ctx.enter_context(tc.tile_pool(name="const", bufs=1))
        sb = ctx.enter_context(tc.tile_pool(name="sb", bufs=4))
        sb6 = ctx.enter_context(tc.tile_pool(name="sb6", bufs=5))
        pp = ctx.enter_context(tc.tile_pool(name="psum", bufs=2, space="PSUM"))

        # ---- constants ----
        def cast_const(name, src, rows, cols, dt=bf16):
            t32 = cpool.tile([rows, cols], fp32, tag=name + "_32", name=name + "_32")
            nc.sync.dma_start(out=t32[:], in_=src)
            tb = cpool.tile([rows, cols], dt, tag=name, name=name)
            nc.vector.tensor_copy(tb[:], t32[:])
            return tb

        def load_const(name, src, rows, cols):
            t = cpool.tile([rows, cols], fp32, tag=name, name=name)
            nc.sync.dma_start(out=t[:], in_=src)
            return t

        wix_b0 = cast_const("wix0", wix[0:104, :], 104, H)
        wie_b = cast_const("wie", wie[:], BOND_FPAD, H)
        wxe_b = cast_const("wxe", wxe[:], 48, H)
        whT_b = [cast_const(f"wh{ci}", whT[H3OFF[ci]:H3OFF[ci] + H3[ci], :],
                            H3[ci], H) for ci in range(3)]
        wox_b0 = cast_const("wox0", wox[0:P, :], P, H)
        wox_b1 = cast_const("wox1", wox[P:ATOM_FPAD, :], ATOM_FPAD - P, H)
        womT_b = [cast_const(f"wom{ci}", womT[H3OFF[ci]:H3OFF[ci] + H3[ci], :],
                             H3[ci], H) for ci in range(3)]
        f1T_b = [cast_const(f"f1_{ci}", f1T[H3OFF[ci]:H3OFF[ci] + H3[ci], :],
                            H3[ci], H) for ci in range(3)]
        f2T_b = [cast_const(f"f2_{ci}", f2T[H3OFF[ci]:H3OFF[ci] + H3[ci], :],
                            H3[ci], 1) for ci in range(3)]
        wob_t = load_const("wob", wob[:], P, H)
        f1b_t = load_const("f1b", f1b[:], P, H)
        f2b_t = load_const("f2b", f2b[:], P, 1)
        iota_t = load_const("iota", iota_in[:], P, P)
        ident_f = load_const("identf", ident_in[:], P, P)
        ident_b = cast_const("identb", ident_in[:], P, P)

        tgt_all = load_const("tgt_all", tgt_pf[:], P, G * 4)
        gsrc_all = cpool.tile([P, G * 4], i32, tag="gsrc_all", name="gsrc_all")
        nc.sync.dma_start(out=gsrc_all[:], in_=gsrc_pf[:])
        xsrc_all = cpool.tile([P, G * 4], i32, tag="xsrc_all", name="xsrc_all")
        nc.sync.dma_start(out=xsrc_all[:], in_=xsrc_pf[:])
        ahf_all = cpool.tile([P, G], i32, tag="ahf_all", name="ahf_all")
        nc.sync.dma_start(out=ahf_all[:], in_=ahf_i[:])
        molf_all = cpool.tile([P, G2], i32, tag="molf_all", name="molf_all")
        nc.sync.dma_start(out=molf_all[:], in_=molf_i[:])
        molpf_all = load_const("molpf_all", mol_pf_in[:], P, G2 * 4)
        xt0_all = cpool.tile([P, G * P], bf16, tag="xt0_all", name="xt0_all")
        nc.sync.dma_start(out=xt0_all[:], in_=xT0_in[:])
        xt1_all = cpool.tile([ATOM_FPAD - P, G * P], bf16, tag="xt1_all",
                             name="xt1_all")
        nc.sync.dma_start(out=xt1_all[:], in_=xT1_in[:])

        def onehot(idx_col, tag):
            s = sb.tile([P, P], bf16, tag=tag)
            nc.vector.tensor_tensor(out=s[:], in0=idx_col.to_broadcast([P, P]),
                                    in1=iota_t[:], op=AO.is_equal)
            return s

        def scatter_mms(neiT_ps, m_slice, s_tile, j):
            # one PSUM bank: start clears has_written BANK-wide, so only the
            # very first matmul of the group may set it (and one stop at end)
            for ci in range(3):
                nc.tensor.matmul(
                    out=neiT_ps[0:H3[ci], P * ci:P * ci + P],
                    lhsT=m_slice[:, H3OFF[ci]:H3OFF[ci] + H3[ci]],
                    rhs=s_tile[:],
                    start=(j == 0 and ci == 0), stop=(j == 3 and ci == 2))

        def dense_from_neiT(neiT_ps, wchunks, extra=()):
            neiT_sb = sb.tile([P, 3 * P], bf16, tag="neiTsb")
            nc.vector.tensor_copy(neiT_sb[:], neiT_ps[:])
            hps = pp.tile([P, H], fp32, space="PSUM", tag="hnei")
            nmm = 3 + len(extra)
            k = 0
            for lhsT, rhs in extra:
                nc.tensor.matmul(out=hps[:], lhsT=lhsT, rhs=rhs,
                                 start=(k == 0), stop=(k == nmm - 1))
                k += 1
            for ci in range(3):
                nc.tensor.matmul(
                    out=hps[:],
                    lhsT=neiT_sb[0:H3[ci], P * ci:P * ci + P],
                    rhs=wchunks[ci][:],
                    start=(k == 0), stop=(k == nmm - 1))
                k += 1
            return hps

        def relu_to(dst_ap, src_ap):
            nc.scalar.activation(out=dst_ap, in_=src_ap,
                                 func=mybir.ActivationFunctionType.Relu)

        # ================= PASS 1 =================
        for g in range(G):
            ea_b = sb.tile([BOND_FPAD, SLOTS], bf16, tag="eab")
            nc.sync.dma_start(out=ea_b[:], in_=eaT_in[g])

            m_bf = sb.tile([P, 4, H], bf16, tag="mbf")
            neiT_ps = pp.tile([P, 3 * P], fp32, space="PSUM", tag="neiT")
            for j in range(4):
                gx = sb.tile([P, ATOM_FPAD], bf16, tag="gx")
                nc.gpsimd.indirect_dma_start(
                    out=gx[:], out_offset=None, in_=x_pad[:],
                    in_offset=bass.IndirectOffsetOnAxis(
                        ap=xsrc_all[:, g * 4 + j:g * 4 + j + 1], axis=0))
                tps = pp.tile([P, 2 * P], bf16, space="PSUM", tag="tps")
                nc.tensor.transpose(out=tps[0:104, 0:P], in_=gx[:, 0:104],
                                    identity=ident_b[:])
                nc.tensor.transpose(out=tps[0:32, P:2 * P],
                                    in_=gx[:, 104:ATOM_FPAD],
                                    identity=ident_b[:])
                xt_b = sb.tile([104, P], bf16, tag="xtb")
                nc.vector.tensor_copy(xt_b[:], tps[0:104, 0:P])
                xc_b = sb.tile([48, P], bf16, tag="xcb")
                nc.vector.tensor_copy(xc_b[0:32, :], tps[0:32, P:2 * P])
                nc.vector.tensor_copy(xc_b[32:48, :], ea_b[:, P * j:P * j + P])
                m0ps = pp.tile([P, H], fp32, space="PSUM", tag="m0")
                nc.tensor.matmul(out=m0ps[:], lhsT=xt_b[:],
                                 rhs=wix_b0[:], start=True, stop=False)
                nc.tensor.matmul(out=m0ps[:], lhsT=xc_b[:],
                                 rhs=wxe_b[:], start=False, stop=True)
                relu_to(m_bf[:, j, :], m0ps[:])
                s = onehot(tgt_all[:, g * 4 + j:g * 4 + j + 1], "s1")
                scatter_mms(neiT_ps, m_bf[:, j, :], s, j)
            nc.sync.dma_start(out=m_dram[g], in_=m_bf[:])
            hps = dense_from_neiT(neiT_ps, whT_b)
            hl_sb = sb.tile([P, H], bf16, tag="hlsb")
            nc.scalar.activation(out=hl_sb[:], in_=hps[:],
                                 func=mybir.ActivationFunctionType.Copy)
            c = g2chunk[g]
            go = g - ch_start[c]
            nc.sync.dma_start(out=hl1[c][go * P:(go + 1) * P, :],
                              in_=hl_sb[:])
            if g == ch_start[c + 1] - 1:
                nc.gpsimd.collective_compute(
                    "AllGather", AO.bypass, replica_groups=RG,
                    ins=[hl1[c][:]],
                    outs=[tab1[ch_row0[c]:ch_row0[c]
                                + NCORES * ch_sizes[c] * P, :]])

        if debug:
            for g in range(G):
                nc.sync.dma_start(out=dbg_m1[g], in_=m_dram[g])
            nc.sync.dma_start(out=dbg_tab1[:], in_=tab1[:])

        # ---- zero ah_tab (uninit DRAM must not feed matmuls); emitted
        # here so the writes hide under pass-1/2 instead of the startup ----
        zero_b = cpool.tile([P, H], bf16, tag="zeros")
        nc.vector.memset(zero_b[:], 0.0)
        r = memset_start
        while r < NA + 512:
            n = min(P, NA + 512 - r)
            nc.sync.dma_start(out=ah_tab[r:r + n, :], in_=zero_b[0:n, :])
            r += n

        # ================= PASS 2 =================
        for g in range(G):
            m_in = sb.tile([P, 4, H], bf16, tag="min")
            nc.sync.dma_start(out=m_in[:], in_=m_dram[g])
            m_bf = sb.tile([P, 4, H], bf16, tag="mbf2")
            gth = sb6.tile([P, 4, H], bf16, tag="gth")
            neiT_ps = pp.tile([P, 3 * P], fp32, space="PSUM", tag="neiT")
            for j in range(4):
                nc.gpsimd.indirect_dma_start(
                    out=gth[:, j, :], out_offset=None, in_=tab1[:],
                    in_offset=bass.IndirectOffsetOnAxis(
                        ap=gsrc_all[:, g * 4 + j:g * 4 + j + 1], axis=0))
                tmp = sb.tile([P, H], fp32, tag="tmp2")
                nc.vector.tensor_tensor(out=tmp[:], in0=m_in[:, j, :],
                                        in1=gth[:, j, :], op=AO.add)
                relu_to(m_bf[:, j, :], tmp[:])
                s = onehot(tgt_all[:, g * 4 + j:g * 4 + j + 1], "s2")
                scatter_mms(neiT_ps, m_bf[:, j, :], s, j)
            nc.sync.dma_start(out=m_dram[g], in_=m_bf[:])
            hps = dense_from_neiT(neiT_ps, whT_b)
            hl_sb = sb.tile([P, H], bf16, tag="hlsb2")
            nc.scalar.activation(out=hl_sb[:], in_=hps[:],
                                 func=mybir.ActivationFunctionType.Copy)
            c = g2chunk[g]
            go = g - ch_start[c]
            nc.sync.dma_start(out=hl2[c][go * P:(go + 1) * P, :],
                              in_=hl_sb[:])
            if g == ch_start[c + 1] - 1:
                nc.gpsimd.collective_compute(
                    "AllGather", AO.bypass, replica_groups=RG,
                    ins=[hl2[c][:]],
                    outs=[tab2[ch_row0[c]:ch_row0[c]
                                + NCORES * ch_sizes[c] * P, :]])

        if debug:
            for g in range(G):
                nc.sync.dma_start(out=dbg_m2[g], in_=m_dram[g])
            nc.sync.dma_start(out=dbg_tab2[:], in_=tab2[:])

        # ================= PASS 3 + atom readout =================
        for g in range(G):
            m_in = sb6.tile([P, 4, H], bf16, tag="min3")
            nc.sync.dma_start(out=m_in[:], in_=m_dram[g])
            msgT_ps = pp.tile([P, 3 * P], fp32, space="PSUM", tag="neiT")
            for j in range(4):
                gth = sb6.tile([P, H], bf16, tag="gth3")
                nc.gpsimd.indirect_dma_start(
                    out=gth[:], out_offset=None, in_=tab2[:],
                    in_offset=bass.IndirectOffsetOnAxis(
                        ap=gsrc_all[:, g * 4 + j:g * 4 + j + 1], axis=0))
                tmp = sb.tile([P, H], fp32, tag="tmp3")
                nc.vector.tensor_tensor(out=tmp[:], in0=m_in[:, j, :],
                                        in1=gth[:], op=AO.add)
                m2 = sb.tile([P, H], bf16, tag="m2")
                relu_to(m2[:], tmp[:])
                s = onehot(tgt_all[:, g * 4 + j:g * 4 + j + 1], "s3")
                scatter_mms(msgT_ps, m2[:], s, j)
            hps = dense_from_neiT(
                msgT_ps, womT_b,
                extra=[(xt0_all[:, g * P:(g + 1) * P], wox_b0[:]),
                       (xt1_all[:, g * P:(g + 1) * P], wox_b1[:])])
            tmp = sb.tile([P, H], fp32, tag="tmpah")
            nc.vector.tensor_tensor(out=tmp[:], in0=hps[:], in1=wob_t[:],
                                    op=AO.add)
            ah_sb = sb.tile([P, H], bf16, tag="ahsb")
            relu_to(ah_sb[:], tmp[:])
            nc.gpsimd.indirect_dma_start(
                out=ah_tab[:],
                out_offset=bass.IndirectOffsetOnAxis(
                    ap=ahf_all[:, g:g + 1], axis=0),
                in_=ah_sb[:], in_offset=None)

        if debug:
            nc.sync.dma_start(out=dbg_ah[:], in_=ah_tab[:])

        # ================= molecule reduction =================
        for g2 in range(G2):
            ah_in = sb.tile([P, 4, H], bf16, tag="ahin")
            nc.sync.dma_start(
                out=ah_in[:],
                in_=ah_tab[g2 * G2SLOTS:(g2 + 1) * G2SLOTS, :].rearrange(
                    "(p j) h -> p j h", j=4))
            mol_ps = pp.tile([P, H], fp32, space="PSUM", tag="hnei")
            for j in range(4):
                s = onehot(molpf_all[:, g2 * 4 + j:g2 * 4 + j + 1], "sm")
                nc.tensor.matmul(out=mol_ps[:], lhsT=s[:], rhs=ah_in[:, j, :],
                                 start=(j == 0), stop=(j == 3))
            mol_sb = sb.tile([P, H], bf16, tag="molsb")
            nc.vector.tensor_copy(mol_sb[:], mol_ps[:])
            nc.gpsimd.indirect_dma_start(
                out=mol_tab[:],
                out_offset=bass.IndirectOffsetOnAxis(
                    ap=molf_all[:, g2:g2 + 1], axis=0),
                in_=mol_sb[:], in_offset=None)

        if debug:
            nc.sync.dma_start(out=dbg_mol[:], in_=mol_tab[:])

        # ================= FFN =================
        for t in range(4):
            mt = sb.tile([P, H], bf16, tag="mt")
            nc.sync.dma_start(out=mt[:], in_=mol_tab[t * P:(t + 1) * P, :])
            hT = sb.tile([P, 3 * P], bf16, tag="hT")
            for ci in range(3):
                tp = pp.tile([P, P], bf16, space="PSUM", tag="tps")
                nc.tensor.transpose(out=tp[0:H3[ci], 0:P],
                                    in_=mt[:, H3OFF[ci]:H3OFF[ci] + H3[ci]],
                                    identity=ident_b[:])
                nc.vector.tensor_copy(hT[:, P * ci:P * ci + P], tp[:, 0:P])
            hps = pp.tile([P, H], fp32, space="PSUM", tag="hnei")
            for ci in range(3):
                nc.tensor.matmul(out=hps[:],
                                 lhsT=hT[0:H3[ci], P * ci:P * ci + P],
                                 rhs=f1T_b[ci][:], start=(ci == 0),
                                 stop=(ci == 2))
            tmp = sb.tile([P, H], fp32, tag="tmpf")
            nc.vector.tensor_tensor(out=tmp[:], in0=hps[:], in1=f1b_t[:],
                                    op=AO.add)
            h_sb = sb.tile([P, H], bf16, tag="hsb")
            relu_to(h_sb[:], tmp[:])
            h2T = sb.tile([P, 3 * P], bf16, tag="h2T")
            for ci in range(3):
                tp = pp.tile([P, P], bf16, space="PSUM", tag="tps")
                nc.tensor.transpose(out=tp[0:H3[ci], 0:P],
                                    in_=h_sb[:, H3OFF[ci]:H3OFF[ci] + H3[ci]],
                                    identity=ident_b[:])
                nc.vector.tensor_copy(h2T[:, P * ci:P * ci + P], tp[:, 0:P])
            ops = pp.tile([P, 1], fp32, space="PSUM", tag="m0")
            for ci in range(3):
                nc.tensor.matmul(out=ops[:],
                                 lhsT=h2T[0:H3[ci], P * ci:P * ci + P],
                                 rhs=f2T_b[ci][:], start=(ci == 0),
                                 stop=(ci == 2))
            o_sb = sb.tile([P, 1], fp32, tag="osb")
            nc.vector.tensor_tensor(out=o_sb[:], in0=ops[:], in1=f2b_t[:],
                                    op=AO.add)
            nc.sync.dma_start(out=out_ext[t * P:(t + 1) * P, :], in_=o_sb[:])

    nc.compile()
    return nc


# ----------------------------------------------------------------------
# entry point
# ----------------------------------------------------------------------

def _build_in_maps(inputs, per_core, meta):
    x = np.asarray(inputs["x"], dtype=np.float32)
    N = x.shape[0]
    x_pad = np.zeros((N, ATOM_FPAD), dtype=ml_dtypes.bfloat16)
    x_pad[:, :ATOM_F] = x.astype(ml_dtypes.bfloat16)

    W_i = np.asarray(inputs["W_i"], dtype=np.float32)
    W_h = np.asarray(inputs["W_h"], dtype=np.float32)
    W_o_w = np.asarray(inputs["W_o_w"], dtype=np.float32)
    W_o_b = np.asarray(inputs["W_o_b"], dtype=np.float32)
    f1w = np.asarray(inputs["ffn1_w"], dtype=np.float32)
    f1b = np.asarray(inputs["ffn1_b"], dtype=np.float32)
    f2w = np.asarray(inputs["ffn2_w"], dtype=np.float32)
    f2b = np.asarray(inputs["ffn2_b"], dtype=np.float32)

    wix = np.zeros((ATOM_FPAD, HIDDEN), np.float32)
    wix[:ATOM_F] = W_i[:, :ATOM_F].T
    wie = np.zeros((BOND_FPAD, HIDDEN), np.float32)
    wie[:BOND_F] = W_i[:, ATOM_F:].T
    wxe = np.zeros((48, HIDDEN), np.float32)
    wxe[:32] = wix[104:ATOM_FPAD]
    wxe[32:32 + BOND_F] = W_i[:, ATOM_F:].T
    wox = np.zeros((ATOM_FPAD, HIDDEN), np.float32)
    wox[:ATOM_F] = W_o_w[:, :ATOM_F].T
    shared = dict(
        x_pad=x_pad,
        iota=np.tile(np.arange(P, dtype=np.float32), (P, 1)),
        ident=np.eye(P, dtype=np.float32),
        wix=wix, wie=wie, wxe=wxe,
        whT=np.ascontiguousarray(W_h.T),
        wox=wox,
        womT=np.ascontiguousarray(W_o_w[:, ATOM_F:].T),
        wob=np.tile(W_o_b[None, :], (P, 1)).astype(np.float32),
        f1T=np.ascontiguousarray(f1w.T),
        f1b=np.tile(f1b[None, :], (P, 1)).astype(np.float32),
        f2T=np.ascontiguousarray(f2w.T),
        f2b=np.full((P, 1), float(f2b[0]), np.float32),
    )
    in_maps = []
    for c in range(NCORES):
        pc = per_core[c]
        m = dict(shared)
        G, G2 = meta["G"], meta["G2"]
        m.update(
            tgt_pf=pc["tgt_pf"].reshape(P, G * 4),
            xsrc_pf=pc["xsrc_pf"].reshape(P, G * 4),
            gsrc_pf=pc["gsrc_pf"].reshape(P, G * 4),
            eaT=pc["eaT"],
            xT0=pc["xT0"].reshape(P, G * P),
            xT1=pc["xT1"].reshape(ATOM_FPAD - P, G * P),
            ahf_i=pc["ah_flush"], molf_i=pc["mol_flush"],
            mol_pf=pc["mol_pf"].reshape(P, G2 * 4),
        )
        in_maps.append(m)
    return in_maps


_CACHED = {}


def kernel(profile=False, debug=False, **inputs):
    from concourse.bass_utils import run_bass_kernel_spmd

    per_core, meta = prep(inputs["x"], inputs["edge_index"],
                          inputs["edge_attr"], inputs["batch"])
    G, G2 = meta["G"], meta["G2"]
    # g2-slot pad rows are interspersed: zero the whole table
    memset_start = 0
    key = (G, G2, memset_start, meta["N"], tuple(meta["ch_sizes"]), debug)
    if key not in _CACHED:
        _CACHED[key] = build_kernel(G, G2, memset_start, meta["N"],
                                    ch_sizes=meta["ch_sizes"], debug=debug)
    nc = _CACHED[key]
    in_maps = _build_in_maps(inputs, per_core, meta)
    res = run_bass_kernel_spmd(nc, in_maps, core_ids=list(range(NCORES)),
                               trace=profile)
    out = np.concatenate([res.results[c]["out"] for c in range(NCORES)],
                         axis=0).astype(np.float32)
    if debug:
        return out, res.results
    if profile:
        return out, res.exec_time_ns
    return out
